# revision 4
# baseline (speedup 1.0000x reference)
"""Block-sparse attention Trainium2 kernel (8 NeuronCores, SPMD).

Problem: hidden_states [2, 2048, 2048] fp32; Wq/Wk/Wv [2048, 2048]; Wo
[2048, 2048]. 16 heads x 128 dim, block-banded attention (BLOCK=64,
bandwidth 2 -> each 128-query tile attends a 384-key band with two
64x64 invalid corners).

Sharding: core c = (batch b = c//4) x (head group g = c%4, 4 heads).
Each core computes q/k/v projections for its 4 heads (columns of
Wq/Wk/Wv), banded attention, and a partial output through its rows of
Wo. Host sums the 4 partials per batch. No collectives.

Per-core pipeline (all matmuls bf16, fp32 PSUM accumulate; inputs are
pre-transposed/cast to bf16 host-side during sharding):
  Inputs land in a few large packed DMAs whose arrival order matches
  PE consumption order (hT quarter-seq chunks; wq/wk packed per head;
  wv/wo k-major), so projections start ~3MB into the load instead of
  waiting for the full 16MB stream.
  QT_h/KT_h produced directly transposed (lhsT=weight slice, rhs=hT);
  V natural [seq, d] (lhsT=hT slice, rhs=Wv).
  scores = QT^T KT band -> +mask tile (fused PSUM->SBUF move) ->
  exp with fused rowsum (no max-subtract; scores are O(+-8)) ->
  reciprocal -> normalize P -> PE-transpose P chunks -> PV -> AO^T bf16.
  out_partial = AO @ Wo_rows via lhsT=AO^T, fused into the last head's
  loop; the last head walks query tiles in order [12..15, 0..11] so the
  final Wo groups depend on long-finished softmax chains (no tail
  bubble). bf16 partials summed in fp32 on host.
rel err ~6e-3 vs the fp32 reference.
"""

from contextlib import ExitStack

import numpy as np

import concourse.bass as bass
import concourse.mybir as mybir
import concourse.tile as tile
from concourse import bacc
from concourse.bass_utils import run_bass_kernel_spmd
from concourse.masks import make_identity

S = 2048          # sequence length
HID = 2048        # hidden size
HL = 4            # heads per core
D = 128           # head dim
NKT = HID // 128  # 16 contraction tiles
NQ = S // 128     # 16 query tiles
SCALE = float(D) ** -0.5
NEG = -1e30
BF = mybir.dt.bfloat16
F32 = mybir.dt.float32


def _emit_wo(nc, ps_big, osb_pool, AO_T, wo_t, out, mt):
    mts = slice(128 * mt, 128 * (mt + 1))
    for nc_ in range(4):
        ns = slice(512 * nc_, 512 * (nc_ + 1))
        ops_ = ps_big.tile([128, 512], mybir.dt.float32, tag="big", name="wops")
        for dk in range(HL):
            nc.tensor.matmul(
                ops_, lhsT=AO_T[dk][:, mts],
                rhs=wo_t[:, 2048 * dk + 512 * nc_ : 2048 * dk + 512 * (nc_ + 1)],
                start=(dk == 0), stop=(dk == HL - 1),
            )
        osb = osb_pool.tile([128, 512], BF, tag="osb", name="osb")
        nc.any.tensor_copy(osb, ops_)
        nc.sync.dma_start(out=out[mts, ns], in_=osb)


def build():
    nc = bacc.Bacc()
    # ht = h^T [hidden, seq]; wqk packs per-head [wq_h | wk_h] column
    # blocks; all inputs pre-transposed/cast to bf16 host-side.
    ht = nc.declare_dram_parameter("ht", [HID, S], BF, isOutput=False)
    wqk = nc.declare_dram_parameter("wqk", [HID, HL * 2 * D], BF, isOutput=False)
    wv = nc.declare_dram_parameter("wv", [HID, HL * D], BF, isOutput=False)
    wo = nc.declare_dram_parameter("wo", [HL * D, HID], BF, isOutput=False)
    out = nc.declare_dram_parameter("out", [S, HID], BF, isOutput=True)

    with ExitStack() as ctx:
        tc = ctx.enter_context(tile.TileContext(nc))
        persist = ctx.enter_context(tc.tile_pool(name="persist", bufs=1))
        qk = ctx.enter_context(tc.tile_pool(name="qk", bufs=2))
        work = ctx.enter_context(tc.tile_pool(name="work", bufs=5))
        stats = ctx.enter_context(tc.tile_pool(name="stats", bufs=8))
        osb_pool = ctx.enter_context(tc.tile_pool(name="osb", bufs=3))
        ps_big = ctx.enter_context(tc.tile_pool(name="ps_big", bufs=4, space="PSUM"))
        ps_sc = ctx.enter_context(tc.tile_pool(name="ps_sc", bufs=1, space="PSUM"))
        ps_pt = ctx.enter_context(tc.tile_pool(name="ps_pt", bufs=2, space="PSUM"))
        ps_ao = ctx.enter_context(tc.tile_pool(name="ps_ao", bufs=1, space="PSUM"))

        # ---- input tiles: packed k-major so a handful of big DMAs with
        # 1-2KB lines feed them in consumption order.
        # hTq[q][:, 512*k : 512*(k+1)] = h^T[128k:128(k+1), 512q:512(q+1)]
        hTq = [persist.tile([128, NKT * 512], BF, tag=f"htq{q}", name=f"htq{q}")
               for q in range(4)]
        # wqk_s[h][:, 256*k : 256*k+128] = Wq rows 128k.., head h cols;
        # [..+128 : ..+256] = Wk same
        wqk_s = [persist.tile([128, NKT * 256], BF, tag=f"wqk{h}", name=f"wqk{h}")
                 for h in range(HL)]
        wv_t = persist.tile([128, NKT * 512], BF, tag="wv", name="wv_t")
        wo_t = persist.tile([128, HL * 2048], BF, tag="wo", name="wo_t")

        # partition dim must stay outermost in the SBUF-side APs; the
        # dram side mirrors the same (p, k, c) element order
        ht_r = ht.rearrange("(k p) (q c) -> q p k c", p=128, c=512)
        wqk_r = wqk.rearrange("(k p) (h c) -> h p k c", p=128, c=256)
        wv_r = wv.rearrange("(k p) c -> p k c", p=128)
        wo_r = wo.rearrange("(k p) c -> p k c", p=128)
        hTq_v = [t.rearrange("p (k c) -> p k c", c=512) for t in hTq]
        wqk_v = [t.rearrange("p (k c) -> p k c", c=256) for t in wqk_s]
        wv_v = wv_t.rearrange("p (k c) -> p k c", c=512)
        wo_v = wo_t.rearrange("p (k c) -> p k c", c=2048)

        # identity before anything else on gpsimd: the PE warm-up loop
        # below depends on it
        ident = persist.tile([128, 128], BF, tag="ident")
        make_identity(nc, ident)

        # ---- DMA issue, priority-ordered per queue.  sync: hT q0,q1;
        # gpsimd (SWDGE): hT q2,q3 + wv + wo; scalar: wqk per head.
        for j in range(4):
            nc.sync.dma_start(out=hTq_v[0][:, 4 * j : 4 * (j + 1)],
                              in_=ht_r[0][:, 4 * j : 4 * (j + 1)])
        for j in range(2):
            nc.sync.dma_start(out=hTq_v[1][:, 8 * j : 8 * (j + 1)],
                              in_=ht_r[1][:, 8 * j : 8 * (j + 1)])
            nc.scalar.dma_start(out=wqk_v[0][:, 8 * j : 8 * (j + 1)],
                                in_=wqk_r[0][:, 8 * j : 8 * (j + 1)])
        for q in (2, 3):
            for j in range(2):
                nc.gpsimd.dma_start(out=hTq_v[q][:, 8 * j : 8 * (j + 1)],
                                    in_=ht_r[q][:, 8 * j : 8 * (j + 1)])
        for h in range(1, HL):
            nc.scalar.dma_start(out=wqk_v[h], in_=wqk_r[h])
        for j in range(2):
            nc.gpsimd.dma_start(out=wv_v[:, 8 * j : 8 * (j + 1)],
                                in_=wv_r[:, 8 * j : 8 * (j + 1)])
        nc.gpsimd.dma_start(out=wo_v, in_=wo_r)

        # additive corner masks for the 384-wide (interior) and 256-wide
        # (edge) score bands; built once (gpsimd, after its DMA issues)
        mask_int = persist.tile([128, 384], F32, tag="mask_int")
        nc.gpsimd.memset(mask_int, 0.0)
        nc.gpsimd.memset(mask_int[0:64, 320:384], NEG)
        nc.gpsimd.memset(mask_int[64:128, 0:64], NEG)
        mask_lo = persist.tile([128, 256], F32, tag="mask_lo")
        nc.gpsimd.memset(mask_lo, 0.0)
        nc.gpsimd.memset(mask_lo[0:64, 192:256], NEG)
        mask_hi = persist.tile([128, 256], F32, tag="mask_hi")
        nc.gpsimd.memset(mask_hi, 0.0)
        nc.gpsimd.memset(mask_hi[64:128, 0:64], NEG)

        # HAM warm-up: dependency-free matmuls at t=0 flip the PE clock
        # gate to 2.4GHz and bridge the DMA-bound window before the
        # first projection chains have data
        warm_ps = ps_ao.tile([128, 128], F32, tag="ao", name="warm_ps")
        for _ in range(48):
            nc.tensor.matmul(warm_ps, lhsT=ident, rhs=ident, start=True, stop=True)

        V = [persist.tile([128, HL * D], BF, tag=f"v{t}", name=f"v{t}") for t in range(NQ)]
        AO_T = [persist.tile([128, S], BF, tag=f"ao{hh}", name=f"ao{hh}") for hh in range(HL)]

        # last head walks qt so the final Wo emissions depend on
        # long-finished AO tiles: compute [12,13,14,15,0..11]; emit
        # Wo(12,13,14,15,0..9) inside the loop (lag 2), Wo(10,11) after.
        qt_tail = [12, 13, 14, 15] + list(range(12))
        emit_at = {2: 12, 3: 13, 4: 14, 5: 15}
        for i in range(6, 16):
            emit_at[i] = i - 6

        for hh in range(HL):
            QT = qk.tile([128, S], BF, tag="q")
            KT = qk.tile([128, S], BF, tag="k")
            for mc in range(4):
                ms = slice(512 * mc, 512 * (mc + 1))
                qps = ps_big.tile([128, 512], F32, tag="big")
                for k in range(NKT):
                    nc.tensor.matmul(
                        qps, lhsT=wqk_s[hh][:, 256 * k : 256 * k + 128],
                        rhs=hTq[mc][:, 512 * k : 512 * (k + 1)],
                        start=(k == 0), stop=(k == NKT - 1),
                    )
                # fold the 1/sqrt(d) scaling into Q
                nc.vector.tensor_scalar_mul(QT[:, ms], qps, SCALE)
                kps = ps_big.tile([128, 512], F32, tag="big")
                for k in range(NKT):
                    nc.tensor.matmul(
                        kps, lhsT=wqk_s[hh][:, 256 * k + 128 : 256 * (k + 1)],
                        rhs=hTq[mc][:, 512 * k : 512 * (k + 1)],
                        start=(k == 0), stop=(k == NKT - 1),
                    )
                nc.vector.tensor_copy(KT[:, ms], kps)

            if hh == 0:
                # V projection, natural layout [seq, 4*128]; placed after
                # head-0 QK so attention can start as early as possible
                for t in range(NQ):
                    vps = ps_big.tile([128, 512], F32, tag="big")
                    for k in range(NKT):
                        nc.tensor.matmul(
                            vps,
                            lhsT=hTq[t // 4][:, 512 * k + 128 * (t % 4) : 512 * k + 128 * (t % 4 + 1)],
                            rhs=wv_t[:, 512 * k : 512 * (k + 1)],
                            start=(k == 0), stop=(k == NKT - 1),
                        )
                    nc.vector.tensor_copy(V[t], vps)

            hs_ = slice(128 * hh, 128 * (hh + 1))
            qt_order = qt_tail if hh == HL - 1 else list(range(NQ))
            for idx, qt in enumerate(qt_order):
                t0 = max(0, 128 * qt - 128)
                t1 = min(S, 128 * qt + 256)
                W = t1 - t0
                scps = ps_sc.tile([128, W], F32, tag="sc")
                nc.tensor.matmul(
                    scps, lhsT=QT[:, 128 * qt : 128 * (qt + 1)], rhs=KT[:, t0:t1],
                    start=True, stop=True,
                )
                sc = work.tile([128, W], F32, tag="scsb")
                mask = mask_lo if qt == 0 else (mask_hi if qt == NQ - 1 else mask_int)
                # copy PSUM->SBUF fused with the corner mask add
                nc.vector.tensor_add(sc, scps, mask)
                # scores are O(+-8) so exp needs no max subtraction
                # (softmax is shift-invariant; fp32 exp is safe here)
                p = work.tile([128, W], BF, tag="p")
                rsum = stats.tile([128, 1], F32, tag="rsum")
                nc.scalar.activation(
                    p, sc, mybir.ActivationFunctionType.Exp,
                    bias=0.0, scale=1.0, accum_out=rsum,
                )
                rcp = stats.tile([128, 1], F32, tag="rcp")
                nc.vector.reciprocal(rcp, rsum)
                nc.vector.tensor_scalar_mul(p, p, rcp)
                aops = ps_ao.tile([128, 128], F32, tag="ao")
                nch = W // 128
                for ci in range(nch):
                    ptps = ps_pt.tile([128, 128], BF, tag="pt")
                    nc.tensor.transpose(
                        ptps, p[:, 128 * ci : 128 * (ci + 1)], ident
                    )
                    pts = work.tile([128, 128], BF, tag="pts")
                    if ci % 2 == 0:
                        nc.vector.tensor_copy(pts, ptps)
                    else:
                        nc.scalar.copy(pts, ptps)
                    tt = t0 // 128 + ci
                    nc.tensor.matmul(
                        aops, lhsT=V[tt][:, hs_], rhs=pts,
                        start=(ci == 0), stop=(ci == nch - 1),
                    )
                nc.scalar.copy(AO_T[hh][:, 128 * qt : 128 * (qt + 1)], aops)

                if hh == HL - 1 and idx in emit_at:
                    _emit_wo(nc, ps_big, osb_pool, AO_T, wo_t, out, emit_at[idx])
        for mt in (10, 11):
            _emit_wo(nc, ps_big, osb_pool, AO_T, wo_t, out, mt)

    if not nc.is_finalized():
        nc.finalize()
    return nc


_NC = None


def _get_nc():
    global _NC
    if _NC is None:
        _NC = build()
    return _NC


def _in_maps(hidden_states, Wq, Wk, Wv, Wo):
    import ml_dtypes

    bf = ml_dtypes.bfloat16
    hs = np.asarray(hidden_states, dtype=np.float32)
    Wq = np.asarray(Wq, dtype=np.float32)
    Wk = np.asarray(Wk, dtype=np.float32)
    Wv = np.asarray(Wv, dtype=np.float32)
    Wo = np.asarray(Wo, dtype=np.float32)
    maps = []
    for c in range(8):
        b, g = divmod(c, 4)
        sl = slice(512 * g, 512 * (g + 1))
        wq_g = Wq[:, sl]  # [2048, 512]
        wk_g = Wk[:, sl]
        # per head h: [wq_h (128) | wk_h (128)] -> [2048, 1024]
        wqk = np.concatenate(
            [np.concatenate([wq_g[:, 128 * h : 128 * (h + 1)],
                             wk_g[:, 128 * h : 128 * (h + 1)]], axis=1)
             for h in range(4)], axis=1)
        maps.append(
            {
                "ht": np.ascontiguousarray(hs[b].T).astype(bf),
                "wqk": np.ascontiguousarray(wqk).astype(bf),
                "wv": np.ascontiguousarray(Wv[:, sl]).astype(bf),
                "wo": np.ascontiguousarray(Wo[sl, :]).astype(bf),
            }
        )
    return maps


def _gather(results):
    outs = [np.asarray(results[c]["out"]).astype(np.float32) for c in range(8)]
    return np.stack(
        [outs[0] + outs[1] + outs[2] + outs[3],
         outs[4] + outs[5] + outs[6] + outs[7]]
    )


def run(in_maps, trace=False, **kw):
    nc = _get_nc()
    return run_bass_kernel_spmd(nc, in_maps, core_ids=list(range(8)), trace=trace, **kw)


def kernel(hidden_states, Wq, Wk, Wv, Wo):
    maps = _in_maps(hidden_states, Wq, Wk, Wv, Wo)
    res = run(maps)
    return _gather(res.results)


# revision 5
# speedup vs baseline: 1.0103x; 1.0103x over previous
"""Block-sparse attention Trainium2 kernel (8 NeuronCores, SPMD).

Problem: hidden_states [2, 2048, 2048] fp32; Wq/Wk/Wv [2048, 2048]; Wo
[2048, 2048]. 16 heads x 128 dim, block-banded attention (BLOCK=64,
bandwidth 2 -> each 128-query tile attends a 384-key band with two
64x64 invalid corners).

Sharding: core c = (batch b = c//4) x (head group g = c%4, 4 heads).
Each core computes q/k/v projections for its 4 heads (columns of
Wq/Wk/Wv), banded attention, and a partial output through its rows of
Wo. Host sums the 4 partials per batch. No collectives.

Per-core pipeline (all matmuls bf16, fp32 PSUM accumulate; inputs are
pre-transposed/cast to bf16 host-side during sharding):
  hT [hid, seq] + weight tiles stream in over HWDGE/SWDGE (issue spread
  across sync/scalar/gpsimd sequencers); all wq tiles are issued before
  all wk tiles and each head's chains run Q(mc0,1), K(mc0,1), Q(mc2,3),
  K(mc2,3) so the first projection chains are never gated on weights
  that arrive late in the stream.
  QT_h/KT_h produced directly transposed (lhsT=weight slice, rhs=hT);
  V natural [seq, d] (lhsT=hT slice, rhs=Wv).
  scores = QT^T KT band -> +mask tile (fused PSUM->SBUF move) ->
  exp with fused rowsum (no max-subtract; scores are O(+-8)) ->
  reciprocal -> normalize P -> PE-transpose P chunks -> PV -> AO^T bf16.
  out_partial = AO @ Wo_rows via lhsT=AO^T, fused into the last head's
  loop; the last head walks query tiles in order [12..15, 0..11] so the
  final Wo groups depend on long-finished softmax chains, and the
  PSUM->SBUF output copies alternate vector/scalar so neither engine
  serializes the tail. bf16 partials summed in fp32 on host.
rel err ~6e-3 vs the fp32 reference.
"""

from contextlib import ExitStack

import numpy as np

import concourse.bass as bass
import concourse.mybir as mybir
import concourse.tile as tile
from concourse import bacc
from concourse.bass_utils import run_bass_kernel_spmd
from concourse.masks import make_identity

S = 2048          # sequence length
HID = 2048        # hidden size
HL = 4            # heads per core
D = 128           # head dim
NKT = HID // 128  # 16 contraction tiles
NQ = S // 128     # 16 query tiles
SCALE = float(D) ** -0.5
NEG = -1e30
BF = mybir.dt.bfloat16
F32 = mybir.dt.float32


def _emit_wo(nc, ps_big, osb_pool, AO_T, wo_s, out, mt):
    mts = slice(128 * mt, 128 * (mt + 1))
    for nc_ in range(4):
        ns = slice(512 * nc_, 512 * (nc_ + 1))
        ops_ = ps_big.tile([128, 512], mybir.dt.float32, tag="big", name="wops")
        for dk in range(HL):
            nc.tensor.matmul(
                ops_, lhsT=AO_T[dk][:, mts], rhs=wo_s[dk][:, ns],
                start=(dk == 0), stop=(dk == HL - 1),
            )
        osb = osb_pool.tile([128, 512], BF, tag="osb", name="osb")
        if nc_ % 2 == 0:
            nc.vector.tensor_copy(osb, ops_)
        else:
            nc.scalar.copy(osb, ops_)
        nc.sync.dma_start(out=out[mts, ns], in_=osb)


def build():
    nc = bacc.Bacc()
    # ht = h^T [hidden, seq]; all inputs pre-transposed/cast to bf16
    # host-side during sharding
    ht = nc.declare_dram_parameter("ht", [HID, S], BF, isOutput=False)
    wq = nc.declare_dram_parameter("wq", [HID, HL * D], BF, isOutput=False)
    wk = nc.declare_dram_parameter("wk", [HID, HL * D], BF, isOutput=False)
    wv = nc.declare_dram_parameter("wv", [HID, HL * D], BF, isOutput=False)
    wo = nc.declare_dram_parameter("wo", [HL * D, HID], BF, isOutput=False)
    out = nc.declare_dram_parameter("out", [S, HID], BF, isOutput=True)

    with ExitStack() as ctx:
        tc = ctx.enter_context(tile.TileContext(nc))
        persist = ctx.enter_context(tc.tile_pool(name="persist", bufs=1))
        qk = ctx.enter_context(tc.tile_pool(name="qk", bufs=2))
        work = ctx.enter_context(tc.tile_pool(name="work", bufs=5))
        stats = ctx.enter_context(tc.tile_pool(name="stats", bufs=8))
        osb_pool = ctx.enter_context(tc.tile_pool(name="osb", bufs=3))
        ps_big = ctx.enter_context(tc.tile_pool(name="ps_big", bufs=4, space="PSUM"))
        ps_sc = ctx.enter_context(tc.tile_pool(name="ps_sc", bufs=1, space="PSUM"))
        ps_pt = ctx.enter_context(tc.tile_pool(name="ps_pt", bufs=2, space="PSUM"))
        ps_ao = ctx.enter_context(tc.tile_pool(name="ps_ao", bufs=1, space="PSUM"))

        ident = persist.tile([128, 128], BF, tag="ident")
        make_identity(nc, ident)

        # additive corner masks for the 384-wide (interior) and 256-wide
        # (edge) score bands; built once
        mask_int = persist.tile([128, 384], F32, tag="mask_int")
        nc.gpsimd.memset(mask_int, 0.0)
        nc.gpsimd.memset(mask_int[0:64, 320:384], NEG)
        nc.gpsimd.memset(mask_int[64:128, 0:64], NEG)
        mask_lo = persist.tile([128, 256], F32, tag="mask_lo")
        nc.gpsimd.memset(mask_lo, 0.0)
        nc.gpsimd.memset(mask_lo[0:64, 192:256], NEG)
        mask_hi = persist.tile([128, 256], F32, tag="mask_hi")
        nc.gpsimd.memset(mask_hi, 0.0)
        nc.gpsimd.memset(mask_hi[64:128, 0:64], NEG)

        # HAM warm-up: ~5us of dependency-free matmuls at t=0 flips the
        # PE clock gate to 2.4GHz before the first real projection MMs
        # (which are DMA-paced and would otherwise run the first ~45us
        # at the cold 1.2GHz K=4/8 state)
        warm_ps = ps_ao.tile([128, 128], F32, tag="ao", name="warm_ps")
        for _ in range(48):
            nc.tensor.matmul(warm_ps, lhsT=ident, rhs=ident, start=True, stop=True)

        # ---- input loads (plain HWDGE DMAs, bf16).  All wq tiles are
        # issued before all wk tiles: the first Q chains need the full
        # wq stream, while wk is only consumed ~14us later.
        hT = [persist.tile([128, S], BF, tag=f"ht{k}", name=f"ht{k}") for k in range(NKT)]
        wq_s = [persist.tile([128, HL * D], BF, tag=f"wq{k}", name=f"wq{k}") for k in range(NKT)]
        wk_s = [persist.tile([128, HL * D], BF, tag=f"wk{k}", name=f"wk{k}") for k in range(NKT)]
        wv_s = [persist.tile([128, HL * D], BF, tag=f"wv{k}", name=f"wv{k}") for k in range(NKT)]
        # half-split hT loads, issue spread across the two HWDGE
        # sequencers (sync + scalar) plus gpsimd for the later weights —
        # DMA issue is ~0.6us per dma_start and serializes per engine
        for k in range(NKT):
            ks = slice(128 * k, 128 * (k + 1))
            nc.sync.dma_start(out=hT[k][:, 0:1024], in_=ht[ks, 0:1024])
            nc.scalar.dma_start(out=wq_s[k], in_=wq[ks, :])
            # wv rides the slower SWDGE stream: V is consumed ~25us in,
            # while wq/wk gate the very first projection groups
            nc.gpsimd.dma_start(out=wv_s[k], in_=wv[ks, :])
        for k in range(NKT):
            ks = slice(128 * k, 128 * (k + 1))
            nc.scalar.dma_start(out=wk_s[k], in_=wk[ks, :])
        for k in range(NKT):
            ks = slice(128 * k, 128 * (k + 1))
            nc.sync.dma_start(out=hT[k][:, 1024:2048], in_=ht[ks, 1024:2048])
        wo_s = [persist.tile([128, HID], BF, tag=f"wo{k}", name=f"wo{k}") for k in range(HL)]
        for k in range(HL):
            nc.gpsimd.dma_start(out=wo_s[k], in_=wo[128 * k : 128 * (k + 1), :])

        V = [persist.tile([128, HL * D], BF, tag=f"v{t}", name=f"v{t}") for t in range(NQ)]

        AO_T = [persist.tile([128, S], BF, tag=f"ao{hh}", name=f"ao{hh}") for hh in range(HL)]

        # last head walks qt so the final Wo emissions depend on
        # long-finished AO tiles: compute [12..15, 0..11]; emit
        # Wo(12,13,14,15,0..9) inside the loop (lag 2), Wo(10,11) after.
        qt_tail = [12, 13, 14, 15] + list(range(12))
        emit_at = {2: 12, 3: 13, 4: 14, 5: 15}
        for i in range(6, 16):
            emit_at[i] = i - 6

        def proj_chain(dst_ps, w_tiles, hh, mc):
            hs_ = slice(128 * hh, 128 * (hh + 1))
            ms = slice(512 * mc, 512 * (mc + 1))
            for k in range(NKT):
                nc.tensor.matmul(
                    dst_ps, lhsT=w_tiles[k][:, hs_], rhs=hT[k][:, ms],
                    start=(k == 0), stop=(k == NKT - 1),
                )

        for hh in range(HL):
            hs_ = slice(128 * hh, 128 * (hh + 1))
            QT = qk.tile([128, S], BF, tag="q")
            KT = qk.tile([128, S], BF, tag="k")
            # Q(mc0,1) first: fed by wq + hT first halves, the earliest
            # arrivals; K(mc0,1) next (wk right behind wq); then the
            # mc2,3 chains that need hT second halves.
            for q_first, mcs in ((True, (0, 1)), (False, (0, 1)),
                                 (True, (2, 3)), (False, (2, 3))):
                for mc in mcs:
                    ms = slice(512 * mc, 512 * (mc + 1))
                    pst = ps_big.tile([128, 512], F32, tag="big")
                    if q_first:
                        proj_chain(pst, wq_s, hh, mc)
                        # fold the 1/sqrt(d) scaling into Q
                        nc.vector.tensor_scalar_mul(QT[:, ms], pst, SCALE)
                    else:
                        proj_chain(pst, wk_s, hh, mc)
                        nc.vector.tensor_copy(KT[:, ms], pst)

            if hh == 0:
                # V projection, natural layout [seq, 4*128]; placed after
                # head-0 QK so attention can start as early as possible
                for t in range(NQ):
                    vps = ps_big.tile([128, 512], F32, tag="big")
                    ts_ = slice(128 * t, 128 * (t + 1))
                    for k in range(NKT):
                        nc.tensor.matmul(
                            vps, lhsT=hT[k][:, ts_], rhs=wv_s[k],
                            start=(k == 0), stop=(k == NKT - 1),
                        )
                    nc.vector.tensor_copy(V[t], vps)

            qt_order = qt_tail if hh == HL - 1 else list(range(NQ))
            for idx, qt in enumerate(qt_order):
                t0 = max(0, 128 * qt - 128)
                t1 = min(S, 128 * qt + 256)
                W = t1 - t0
                scps = ps_sc.tile([128, W], F32, tag="sc")
                nc.tensor.matmul(
                    scps, lhsT=QT[:, 128 * qt : 128 * (qt + 1)], rhs=KT[:, t0:t1],
                    start=True, stop=True,
                )
                sc = work.tile([128, W], F32, tag="scsb")
                mask = mask_lo if qt == 0 else (mask_hi if qt == NQ - 1 else mask_int)
                # copy PSUM->SBUF fused with the corner mask add
                nc.vector.tensor_add(sc, scps, mask)
                # scores are O(+-8) so exp needs no max subtraction
                # (softmax is shift-invariant; fp32 exp is safe here)
                p = work.tile([128, W], BF, tag="p")
                rsum = stats.tile([128, 1], F32, tag="rsum")
                nc.scalar.activation(
                    p, sc, mybir.ActivationFunctionType.Exp,
                    bias=0.0, scale=1.0, accum_out=rsum,
                )
                rcp = stats.tile([128, 1], F32, tag="rcp")
                nc.vector.reciprocal(rcp, rsum)
                nc.vector.tensor_scalar_mul(p, p, rcp)
                aops = ps_ao.tile([128, 128], F32, tag="ao")
                nch = W // 128
                for ci in range(nch):
                    ptps = ps_pt.tile([128, 128], BF, tag="pt")
                    nc.tensor.transpose(
                        ptps, p[:, 128 * ci : 128 * (ci + 1)], ident
                    )
                    pts = work.tile([128, 128], BF, tag="pts")
                    if ci % 2 == 0:
                        nc.vector.tensor_copy(pts, ptps)
                    else:
                        nc.scalar.copy(pts, ptps)
                    tt = t0 // 128 + ci
                    nc.tensor.matmul(
                        aops, lhsT=V[tt][:, hs_], rhs=pts,
                        start=(ci == 0), stop=(ci == nch - 1),
                    )
                nc.scalar.copy(AO_T[hh][:, 128 * qt : 128 * (qt + 1)], aops)

                if hh == HL - 1 and idx in emit_at:
                    _emit_wo(nc, ps_big, osb_pool, AO_T, wo_s, out, emit_at[idx])
        for mt in (10, 11):
            _emit_wo(nc, ps_big, osb_pool, AO_T, wo_s, out, mt)

    if not nc.is_finalized():
        nc.finalize()
    return nc


_NC = None


def _get_nc():
    global _NC
    if _NC is None:
        _NC = build()
    return _NC


def _in_maps(hidden_states, Wq, Wk, Wv, Wo):
    import ml_dtypes

    bf = ml_dtypes.bfloat16
    hs = np.asarray(hidden_states, dtype=np.float32)
    Wq = np.asarray(Wq, dtype=np.float32)
    Wk = np.asarray(Wk, dtype=np.float32)
    Wv = np.asarray(Wv, dtype=np.float32)
    Wo = np.asarray(Wo, dtype=np.float32)
    maps = []
    for c in range(8):
        b, g = divmod(c, 4)
        sl = slice(512 * g, 512 * (g + 1))
        maps.append(
            {
                "ht": np.ascontiguousarray(hs[b].T).astype(bf),
                "wq": np.ascontiguousarray(Wq[:, sl]).astype(bf),
                "wk": np.ascontiguousarray(Wk[:, sl]).astype(bf),
                "wv": np.ascontiguousarray(Wv[:, sl]).astype(bf),
                "wo": np.ascontiguousarray(Wo[sl, :]).astype(bf),
            }
        )
    return maps


def _gather(results):
    outs = [np.asarray(results[c]["out"]).astype(np.float32) for c in range(8)]
    return np.stack(
        [outs[0] + outs[1] + outs[2] + outs[3],
         outs[4] + outs[5] + outs[6] + outs[7]]
    )


def run(in_maps, trace=False, **kw):
    nc = _get_nc()
    return run_bass_kernel_spmd(nc, in_maps, core_ids=list(range(8)), trace=trace, **kw)


def kernel(hidden_states, Wq, Wk, Wv, Wo):
    maps = _in_maps(hidden_states, Wq, Wk, Wv, Wo)
    res = run(maps)
    return _gather(res.results)
